# revision 28
# baseline (speedup 1.0000x reference)
"""Trainium2 Bass kernel for nn_AutoEncoderLoss (two-level segment-mean MSE).

Strategy
--------
The loss only needs per-(batch, cluster) sums of (reco-target)^2 and counts.
Counts and the grouping are a pure function of the index tensors, so the host
computes the layout: a stable argsort by fused segment id s = b*C + c places
every segment's points contiguously; each segment is padded up to a whole
number of 128-point columns and laid out as a [128, w_s] column block.
Segments are sharded 512-per-core (4 whole batches per core, matching the
data-parallel hint) and concatenated along the free dim into a [128, T] tile
per core (~6% padding overhead), stored tile-major so every DMA is a fully
contiguous 256 KB block.

The device does all the O(N) math: stream rec/tar in 2048-column chunks
(wide per-partition DMA lines - descriptor efficiency - on the two hardware
DGE queues, SP + Activation), d = rec - tar (DVE), v = d^2 (DVE 2x-mode bf16
multiply), then per-column partition sums on the PE: matmul with stationary
= a 128-column v block and moving = a ones vector, giving [128, 1] outputs
(each original column's sum lands on its own partition). Copies to SBUF run
on the Scalar engine and are 128-way parallel; one [128, T/128] f32 DMA out
per core.

The host then folds each segment's w_s column sums (cumsum-diff, float64),
takes counts from the same bincount that defined the layout, and runs the
reference's O(B*C) masked two-level mean. Zero padding is exact: pad slots
have rec = tar = 0 so they contribute 0 to every sum.

Values are quantized to fp8 e4m3 on the host (IN_DTYPE="f8"): the loss is
a mean over ~2048-sample groups with a 2e-2 tolerance, and e4m3 input
rounding adds only ~1.3e-3 relative error (measured) while cutting DMA to
1 byte/value. Chunks are 4096 columns (4 KB DMA lines) and the square
alternates between DVE (2x-mode bf16 multiply) and ScalarE (Square
activation) per chunk ("mix"), balancing engine queues. Measured ~15-19 us
steady state per core (vs 326 us for the staged one-hot-matmul baseline).
"""

import os as _os
import numpy as np
from contextlib import ExitStack

NCORES = 8
B_HC = 32            # hardcoded problem shape (asserted at runtime)
C_HC = 128
SEG_PER_CORE = B_HC * C_HC // NCORES  # 512
TW = 512             # columns per tile = one PSUM bank of f32 column sums
IN_DTYPE = _os.environ.get("K_IN_DTYPE", "f8")  # "f32" | "bf16" | "f8"
DMA_SPLIT = int(_os.environ.get("K_DMA_SPLIT", "2"))  # input DMA trigger engines
SQ_ENGINE = _os.environ.get("K_SQ", "mix")   # square: "dve" | "act" | "mix"
COPY_ENGINE = _os.environ.get("K_COPY", "act")  # psum->sbuf copy engine
LAYOUT = _os.environ.get("K_LAYOUT", "big")  # "big" | "tiles"
PSGROUP = int(_os.environ.get("K_PSGROUP", "8"))  # chunk width in 512-col units
OUTT = int(_os.environ.get("K_OUTT", "1"))  # transposed colsum output [128, T/128]

_prog_cache = {}
_last_run = {}


def _build_program(n_tiles, repeat=None, internal_inputs=False, stage="full"):
    """Build + compile the SPMD bass program over T = n_tiles*TW columns.

    repeat: wrap the compute in a hardware For_i loop (profiling).
    internal_inputs: inputs become Internal DRAM scratch (no host transfer);
    timing is data-independent, used only for profiling.
    stage: "dma" | "sub" | "sq" | "mm" | "full" - truncate the pipeline
    after that stage (engine attribution without perfetto).

    LAYOUT "big": per-core inputs are plain [128, T] arrays; one whole-tensor
    DMA per input per iteration (max per-partition line size = best DMA
    descriptor efficiency), single big sub/mul instructions, matmuls batched
    into PSGROUP-bank PSUM groups with one copy per group.
    LAYOUT "tiles": the original [n_tiles, 128, TW] tile-streamed pipeline.
    """
    import concourse.tile as tile
    from concourse import bacc, mybir

    f32 = mybir.dt.float32
    bf16 = mybir.dt.bfloat16
    AT = mybir.ActivationFunctionType
    in_dt = {"f32": f32, "bf16": bf16, "f8": mybir.dt.float8e4}[IN_DTYPE]
    d_dt = bf16 if IN_DTYPE == "f8" else in_dt
    T = n_tiles * TW

    nc = bacc.Bacc("TRN2", target_bir_lowering=False, debug=False,
                   num_devices=NCORES)
    in_kind = "Internal" if internal_inputs else "ExternalInput"
    in_shape = [128, T] if LAYOUT == "big" else [n_tiles * 128, TW]
    rec = nc.dram_tensor("rec", in_shape, in_dt, kind=in_kind).ap()
    tar = nc.dram_tensor("tar", in_shape, in_dt, kind=in_kind).ap()
    out_shape = [128, T // 128] if OUTT else [1, T]
    out = nc.dram_tensor("out", out_shape, f32, kind="ExternalOutput").ap()

    with tile.TileContext(nc) as tc, ExitStack() as ctx:
        io_pool = ctx.enter_context(tc.tile_pool(name="io", bufs=3))
        tmp_pool = ctx.enter_context(tc.tile_pool(name="tmp", bufs=3))
        one_pool = ctx.enter_context(tc.tile_pool(name="one", bufs=1))
        ps_bufs = 4 if OUTT else (max(1, 8 // PSGROUP) if LAYOUT == "big" else 4)
        psum_pool = ctx.enter_context(tc.tile_pool(name="ps", bufs=ps_bufs,
                                                   space="PSUM"))
        out_pool = ctx.enter_context(tc.tile_pool(name="ob", bufs=1))

        ones = one_pool.tile([128, 1], bf16, tag="ones")
        nc.vector.memset(ones[:], 1.0)
        ob = out_pool.tile([128, T // 128] if OUTT else [1, T], f32, tag="ob")

        if repeat is not None:
            ctx.enter_context(tc.For_i(0, repeat, 1))

        def copy_out(dst_slice, ps_slice):
            if COPY_ENGINE == "act":
                nc.scalar.copy(dst_slice, ps_slice)
            else:
                nc.vector.tensor_copy(dst_slice, ps_slice)

        if LAYOUT == "big":
            # chunks of PSGROUP*TW columns from the plain [128, T] input:
            # wide per-partition DMA lines (descriptor efficiency) while
            # keeping several chunks in flight via the rotating pools
            W = PSGROUP * TW
            for c0 in range(0, T, W):
                cw = min(W, T - c0)
                rec_t = io_pool.tile([128, W], in_dt, tag="rec")
                tar_t = io_pool.tile([128, W], in_dt, tag="tar")
                nc.sync.dma_start(out=rec_t[:, :cw], in_=rec[:, c0:c0 + cw])
                nc.scalar.dma_start(out=tar_t[:, :cw], in_=tar[:, c0:c0 + cw])
                if stage == "dma":
                    continue
                d_t = tmp_pool.tile([128, W], d_dt, tag="d")
                nc.vector.tensor_sub(d_t[:, :cw], rec_t[:, :cw], tar_t[:, :cw])
                if stage == "sub":
                    continue
                v_t = tmp_pool.tile([128, W], bf16, tag="v")
                sq_eng = SQ_ENGINE if SQ_ENGINE != "mix" else \
                    ("dve" if (c0 // W) % 2 else "act")
                if sq_eng == "dve":
                    # bf16 tensor_tensor runs in the DVE 2x mode
                    nc.vector.tensor_mul(v_t[:, :cw], d_t[:, :cw], d_t[:, :cw])
                else:
                    nc.scalar.activation(v_t[:, :cw], d_t[:, :cw], AT.Square)
                if stage == "sq":
                    continue
                if OUTT:
                    # stationary = v 128-col block, moving = ones: out [128,1]
                    # column sums land across all 128 partitions, so the
                    # PSUM->SBUF copy is 128-way parallel (free size W/128)
                    psT = psum_pool.tile([128, W // 128], f32, tag="ps")
                    for j in range(cw // 128):
                        nc.tensor.matmul(psT[:, j:j + 1],
                                         v_t[:, j * 128:(j + 1) * 128],
                                         ones[:], start=True, stop=True)
                    if stage == "mm":
                        continue
                    copy_out(ob[:, c0 // 128:(c0 + cw) // 128],
                             psT[:, :cw // 128])
                else:
                    ps = psum_pool.tile([1, W], f32, tag="ps")
                    for j in range(cw // TW):
                        nc.tensor.matmul(
                            ps[:, j * TW:(j + 1) * TW], ones[:],
                            v_t[:, j * TW:(j + 1) * TW],
                            start=True, stop=True)
                    if stage == "mm":
                        continue
                    copy_out(ob[:, c0:c0 + cw], ps[:, :cw])
        else:
            # input DMAs spread over the two hardware DGE queues (SP + Act)
            for t in range(n_tiles):
                rec_t = io_pool.tile([128, TW], in_dt, tag="rec")
                tar_t = io_pool.tile([128, TW], in_dt, tag="tar")
                rb = rec[t * 128:(t + 1) * 128, :]
                tb = tar[t * 128:(t + 1) * 128, :]
                if DMA_SPLIT == 1:
                    nc.sync.dma_start(out=rec_t[:], in_=rb)
                    nc.sync.dma_start(out=tar_t[:], in_=tb)
                else:
                    nc.sync.dma_start(out=rec_t[:], in_=rb)
                    nc.scalar.dma_start(out=tar_t[:], in_=tb)
                if stage == "dma":
                    continue
                d_t = tmp_pool.tile([128, TW], d_dt, tag="d")
                nc.vector.tensor_sub(d_t[:], rec_t[:], tar_t[:])
                if stage == "sub":
                    continue
                v_t = tmp_pool.tile([128, TW], bf16, tag="v")
                if SQ_ENGINE == "dve":
                    nc.vector.tensor_mul(v_t[:], d_t[:], d_t[:])
                else:
                    nc.scalar.activation(v_t[:], d_t[:], AT.Square)
                if stage == "sq":
                    continue
                ps = psum_pool.tile([1, TW], f32, tag="ps")
                nc.tensor.matmul(ps[:], ones[:], v_t[:], start=True, stop=True)
                if stage == "mm":
                    continue
                copy_out(ob[:, t * TW:(t + 1) * TW], ps[:])

        if stage == "full":
            nc.sync.dma_start(out=out[:], in_=ob[:])

    nc.compile()
    return nc


def kernel(reco, target, clabel, batch_index, num_batches, num_clusters):
    from concourse.bass_utils import run_bass_kernel_spmd

    B = int(num_batches)
    C = int(num_clusters)
    assert B == B_HC and C == C_HC, f"kernel hardcoded for B=32,C=128, got {B},{C}"
    nseg = B * C

    rec = np.ascontiguousarray(np.asarray(reco, dtype=np.float32).reshape(-1))
    tar = np.ascontiguousarray(np.asarray(target, dtype=np.float32).reshape(-1))
    cl = np.asarray(clabel).astype(np.int32).reshape(-1)
    bi = np.asarray(batch_index).astype(np.int32).reshape(-1)
    N = rec.shape[0]

    # host layout: group points by fused segment id (stable counting order)
    key = bi * np.int32(C) + cl                      # [N] in [0, 4096)
    order = np.argsort(key, kind="stable")
    key_s = key[order]
    counts = np.bincount(key, minlength=nseg).astype(np.int64)  # [B*C]
    w = (counts + 127) >> 7                          # columns per segment
    w_pc = w.reshape(NCORES, SEG_PER_CORE)
    colbase_pc = np.zeros((NCORES, SEG_PER_CORE), dtype=np.int64)
    colbase_pc[:, 1:] = np.cumsum(w_pc[:, :-1], axis=1)
    T_core = w_pc.sum(axis=1)                        # used cols per core
    n_tiles = max(1, int(-(-int(T_core.max()) // TW)))
    T = n_tiles * TW

    # destination slot of each (sorted) point: core, partition, column
    seg_start = np.zeros(nseg + 1, dtype=np.int64)
    seg_start[1:] = np.cumsum(counts)
    i_loc = np.arange(N, dtype=np.int64) - seg_start[key_s]
    p = i_loc & 127
    col = colbase_pc.reshape(-1)[key_s] + (i_loc >> 7)
    core = key_s >> np.int64(SEG_PER_CORE.bit_length() - 1)  # key_s // 512
    if LAYOUT == "big":
        # plain [128, T] per core
        dest = (core * 128 + p) * T + col
    else:
        # tile-major per-core layout: [n_tiles, 128, TW] flattened
        dest = (core * (n_tiles * 128) + (col // TW) * 128 + p) * TW + (col % TW)

    if IN_DTYPE == "f32":
        rec_buf = np.zeros(NCORES * n_tiles * 128 * TW, dtype=np.float32)
        tar_buf = np.zeros_like(rec_buf)
        rec_buf[dest] = rec[order]
        tar_buf[dest] = tar[order]
    else:
        import ml_dtypes
        cdt = ml_dtypes.bfloat16 if IN_DTYPE == "bf16" else ml_dtypes.float8_e4m3
        udt = np.uint16 if IN_DTYPE == "bf16" else np.uint8
        rec_buf = np.zeros(NCORES * n_tiles * 128 * TW, dtype=udt)
        tar_buf = np.zeros_like(rec_buf)
        rec_buf[dest] = rec[order].astype(cdt).view(udt)
        tar_buf[dest] = tar[order].astype(cdt).view(udt)
        rec_buf = rec_buf.view(cdt)
        tar_buf = tar_buf.view(cdt)
    in_shape = (128, T) if LAYOUT == "big" else (n_tiles * 128, TW)
    rec_buf = rec_buf.reshape(NCORES, *in_shape)
    tar_buf = tar_buf.reshape(NCORES, *in_shape)

    key_cache = (n_tiles, IN_DTYPE, DMA_SPLIT, SQ_ENGINE, COPY_ENGINE, LAYOUT, PSGROUP, OUTT)
    if key_cache not in _prog_cache:
        _prog_cache[key_cache] = _build_program(n_tiles)
    nc = _prog_cache[key_cache]

    in_maps = [{"rec": rec_buf[m], "tar": tar_buf[m]} for m in range(NCORES)]
    _last_run["nc"] = nc
    _last_run["in_maps"] = in_maps
    _last_run["key"] = key_cache
    _last_run["n_tiles"] = n_tiles

    res = None
    last_err = None
    for _attempt in range(3):  # the device occasionally faults transiently
        try:
            res = run_bass_kernel_spmd(nc, in_maps, list(range(NCORES)))
            break
        except Exception as e:  # noqa: BLE001
            last_err = e
            import time as _time
            _time.sleep(2.0)
    if res is None:
        raise last_err

    # host: fold each segment's column sums, then the O(B*C) final reduction
    sums = np.zeros(nseg, dtype=np.float64)
    for m in range(NCORES):
        o = res.results[m]["out"].astype(np.float64)
        colsums = o.T.reshape(-1) if OUTT else o.reshape(-1)  # [T]
        cs = np.zeros(T + 1, dtype=np.float64)
        cs[1:] = np.cumsum(colsums)
        s0, s1 = colbase_pc[m], colbase_pc[m] + w_pc[m]
        sums[m * SEG_PER_CORE:(m + 1) * SEG_PER_CORE] = cs[s1] - cs[s0]

    counts_f = counts.astype(np.float64).reshape(B, C)
    sums2 = sums.reshape(B, C)
    present = counts_f > 0
    means = np.where(present, sums2 / np.where(present, counts_f, 1.0), 0.0)
    pmask = present.astype(np.float64)
    n_clusters_b = pmask.sum(axis=1)
    b_present = n_clusters_b > 0
    batch_loss = (means * pmask).sum(axis=1) / np.where(b_present, n_clusters_b, 1.0)
    n_b = b_present.sum()
    loss = np.where(b_present, batch_loss, 0.0).sum() / max(n_b, 1)
    return np.float32(loss)


def profile_hw(np_inputs=None, k1=4, k2=4004, pairs=12, verbose=False):
    """Measure steady-state HW ns per kernel iteration.

    Runs two hardware-loop variants (k1/k2 repeats of the full compute,
    Internal-DRAM inputs so no transfers) in interleaved pairs; the median
    of per-pair wall-clock differences divided by (k2-k1) cancels dispatch
    overhead and is robust to the time-shared device's slow patches.
    """
    import time
    from concourse.bass_utils import run_bass_kernel_spmd
    if not _last_run and np_inputs is not None:
        kernel(**np_inputs)
    n_tiles = _last_run["n_tiles"]

    ncs = {}
    for k in (k1, k2):
        ck = ("prof", n_tiles, k, "full", IN_DTYPE, DMA_SPLIT, SQ_ENGINE, COPY_ENGINE, LAYOUT, PSGROUP, OUTT)
        if ck not in _prog_cache:
            _prog_cache[ck] = _build_program(n_tiles, repeat=k,
                                             internal_inputs=True)
        ncs[k] = _prog_cache[ck]

    def one(k):
        t0 = time.time()
        run_bass_kernel_spmd(ncs[k], [{} for _ in range(NCORES)],
                             list(range(NCORES)))
        return time.time() - t0

    one(k1)  # warm both NEFFs
    one(k2)
    t1s, t2s = [], []
    for _ in range(pairs):
        try:
            t1s.append(one(k1))
            t2s.append(one(k2))
        except Exception:  # transient device flake: skip pair
            time.sleep(2)
            continue
    if not t1s or not t2s:
        return float("nan")
    # min-min: the device is time-shared, so contention only ever adds
    # time; the minimum over repeats of each loop length estimates the
    # uncontended steady state.
    minmin = (min(t2s) - min(t1s)) / (k2 - k1) * 1e9
    if verbose:
        diffs = sorted((b - a) / (k2 - k1) * 1e9 for a, b in zip(t1s, t2s))
        print("pair diffs (ns/iter):", [f"{d:.0f}" for d in diffs])
        print(f"median-of-pairs: {diffs[len(diffs) // 2]:.0f} ns")
    return minmin


def profile_stages(np_inputs=None, k1=4, k2=1004, pairs=6):
    """Per-stage steady-state times (us): dma, +sub, +sq, +mm, full.

    Same interleaved-pair median methodology as profile_hw, per stage.
    """
    import time
    from concourse.bass_utils import run_bass_kernel_spmd
    if not _last_run and np_inputs is not None:
        kernel(**np_inputs)
    n_tiles = _last_run["n_tiles"]
    out = {}
    for stage in ("dma", "sub", "sq", "mm", "full"):
        ncs = {}
        for k in (k1, k2):
            ck = ("prof", n_tiles, k, stage, IN_DTYPE, DMA_SPLIT, SQ_ENGINE, COPY_ENGINE, LAYOUT, PSGROUP, OUTT)
            if ck not in _prog_cache:
                _prog_cache[ck] = _build_program(n_tiles, repeat=k,
                                                 internal_inputs=True,
                                                 stage=stage)
            ncs[k] = _prog_cache[ck]

        def one(k):
            t0 = time.time()
            run_bass_kernel_spmd(ncs[k], [{} for _ in range(NCORES)],
                                 list(range(NCORES)))
            return time.time() - t0

        one(k1)
        one(k2)
        diffs = []
        for _ in range(pairs):
            try:
                ta = one(k1)
                tb = one(k2)
            except Exception:
                time.sleep(2)
                continue
            diffs.append((tb - ta) / (k2 - k1) * 1e6)
        diffs.sort()
        out[stage] = diffs[len(diffs) // 2] if diffs else float("nan")
    return out


# revision 30
# speedup vs baseline: 1.1498x; 1.1498x over previous
"""Trainium2 Bass kernel for nn_AutoEncoderLoss (two-level segment-mean MSE).

Strategy
--------
The loss only needs per-(batch, cluster) sums of (reco-target)^2 and counts.
Counts and the grouping are a pure function of the index tensors, so the host
computes the layout: a stable argsort by fused segment id s = b*C + c places
every segment's points contiguously; each segment is padded up to a whole
number of 128-point columns and laid out as a [128, w_s] column block.
Segments are sharded 512-per-core (4 whole batches per core, matching the
data-parallel hint) and concatenated along the free dim into a [128, T] tile
per core (~6% padding overhead), stored tile-major so every DMA is a fully
contiguous 256 KB block.

The device does all the O(N) math: stream rec/tar in 2048-column chunks
(wide per-partition DMA lines - descriptor efficiency - on the two hardware
DGE queues, SP + Activation), d = rec - tar (DVE), v = d^2 (DVE 2x-mode bf16
multiply), then per-column partition sums on the PE: matmul with stationary
= a 128-column v block and moving = a ones vector, giving [128, 1] outputs
(each original column's sum lands on its own partition). Copies to SBUF run
on the Scalar engine and are 128-way parallel; one [128, T/128] f32 DMA out
per core.

The host then folds each segment's w_s column sums (cumsum-diff, float64),
takes counts from the same bincount that defined the layout, and runs the
reference's O(B*C) masked two-level mean. Zero padding is exact: pad slots
have rec = tar = 0 so they contribute 0 to every sum.

Values are quantized to fp8 e4m3 on the host (IN_DTYPE="f8"): the loss is
a mean over ~2048-sample groups with a 2e-2 tolerance, and e4m3 input
rounding adds only ~1.3e-3 relative error (measured) while cutting DMA to
1 byte/value. Chunks are 4096 columns (4 KB DMA lines) and the square
alternates between DVE (2x-mode bf16 multiply) and ScalarE (Square
activation) per chunk ("mix"), balancing engine queues. Measured ~15-19 us
steady state per core (vs 326 us for the staged one-hot-matmul baseline).
"""

import os as _os
import numpy as np
from contextlib import ExitStack

NCORES = 8
B_HC = 32            # hardcoded problem shape (asserted at runtime)
C_HC = 128
SEG_PER_CORE = B_HC * C_HC // NCORES  # 512
TW = 512             # columns per tile = one PSUM bank of f32 column sums
IN_DTYPE = _os.environ.get("K_IN_DTYPE", "f8")  # "f32" | "bf16" | "f8"
DMA_SPLIT = int(_os.environ.get("K_DMA_SPLIT", "2"))  # input DMA trigger engines
SQ_ENGINE = _os.environ.get("K_SQ", "mix")   # square: "dve" | "act" | "mix"
COPY_ENGINE = _os.environ.get("K_COPY", "act")  # psum->sbuf copy engine
LAYOUT = _os.environ.get("K_LAYOUT", "big")  # "big" | "tiles"
PSGROUP = int(_os.environ.get("K_PSGROUP", "8"))  # chunk width in 512-col units
OUTT = int(_os.environ.get("K_OUTT", "1"))  # transposed colsum output [128, T/128]
PREFETCH = int(_os.environ.get("K_PREFETCH", "0"))  # issue all input DMAs first
SUB_ENGINE = _os.environ.get("K_SUB", "dve")  # "dve" | "gp1" (gpsimd mid chunk)

_prog_cache = {}
_last_run = {}


def _build_program(n_tiles, repeat=None, internal_inputs=False, stage="full"):
    """Build + compile the SPMD bass program over T = n_tiles*TW columns.

    repeat: wrap the compute in a hardware For_i loop (profiling).
    internal_inputs: inputs become Internal DRAM scratch (no host transfer);
    timing is data-independent, used only for profiling.
    stage: "dma" | "sub" | "sq" | "mm" | "full" - truncate the pipeline
    after that stage (engine attribution without perfetto).

    LAYOUT "big": per-core inputs are plain [128, T] arrays; one whole-tensor
    DMA per input per iteration (max per-partition line size = best DMA
    descriptor efficiency), single big sub/mul instructions, matmuls batched
    into PSGROUP-bank PSUM groups with one copy per group.
    LAYOUT "tiles": the original [n_tiles, 128, TW] tile-streamed pipeline.
    """
    import concourse.tile as tile
    from concourse import bacc, mybir

    f32 = mybir.dt.float32
    bf16 = mybir.dt.bfloat16
    AT = mybir.ActivationFunctionType
    in_dt = {"f32": f32, "bf16": bf16, "f8": mybir.dt.float8e4}[IN_DTYPE]
    d_dt = bf16 if IN_DTYPE == "f8" else in_dt
    T = n_tiles * TW

    nc = bacc.Bacc("TRN2", target_bir_lowering=False, debug=False,
                   num_devices=NCORES)
    in_kind = "Internal" if internal_inputs else "ExternalInput"
    in_shape = [128, T] if LAYOUT == "big" else [n_tiles * 128, TW]
    rec = nc.dram_tensor("rec", in_shape, in_dt, kind=in_kind).ap()
    tar = nc.dram_tensor("tar", in_shape, in_dt, kind=in_kind).ap()
    out_shape = [128, T // 128] if OUTT else [1, T]
    out = nc.dram_tensor("out", out_shape, f32, kind="ExternalOutput").ap()

    n_chunks = -(-T // (PSGROUP * TW)) if LAYOUT == "big" else 0
    io_bufs = n_chunks if (LAYOUT == "big" and PREFETCH) else 3
    with tile.TileContext(nc) as tc, ExitStack() as ctx:
        io_pool = ctx.enter_context(tc.tile_pool(name="io", bufs=io_bufs))
        tmp_pool = ctx.enter_context(tc.tile_pool(name="tmp", bufs=3))
        one_pool = ctx.enter_context(tc.tile_pool(name="one", bufs=1))
        ps_bufs = 4 if OUTT else (max(1, 8 // PSGROUP) if LAYOUT == "big" else 4)
        psum_pool = ctx.enter_context(tc.tile_pool(name="ps", bufs=ps_bufs,
                                                   space="PSUM"))
        out_pool = ctx.enter_context(tc.tile_pool(name="ob", bufs=1))

        ones = one_pool.tile([128, 1], bf16, tag="ones")
        nc.vector.memset(ones[:], 1.0)
        ob = out_pool.tile([128, T // 128] if OUTT else [1, T], f32, tag="ob")

        if repeat is not None:
            ctx.enter_context(tc.For_i(0, repeat, 1))

        def copy_out(dst_slice, ps_slice):
            if COPY_ENGINE == "act":
                nc.scalar.copy(dst_slice, ps_slice)
            else:
                nc.vector.tensor_copy(dst_slice, ps_slice)

        if LAYOUT == "big":
            # chunks of PSGROUP*TW columns from the plain [128, T] input:
            # wide per-partition DMA lines (descriptor efficiency) while
            # keeping several chunks in flight via the rotating pools
            W = PSGROUP * TW
            chunks = [(c0, min(W, T - c0)) for c0 in range(0, T, W)]
            pre = []
            if PREFETCH:
                # issue every chunk's input DMAs before any compute so no
                # trigger sits behind compute in an engine's queue
                for (c0, cw) in chunks:
                    rec_t = io_pool.tile([128, W], in_dt, tag="rec")
                    tar_t = io_pool.tile([128, W], in_dt, tag="tar")
                    nc.sync.dma_start(out=rec_t[:, :cw], in_=rec[:, c0:c0 + cw])
                    q = nc.scalar if DMA_SPLIT == 2 else nc.sync
                    q.dma_start(out=tar_t[:, :cw], in_=tar[:, c0:c0 + cw])
                    pre.append((rec_t, tar_t))
            for ci, (c0, cw) in enumerate(chunks):
                if PREFETCH:
                    rec_t, tar_t = pre[ci]
                    if stage == "dma":
                        continue
                else:
                    rec_t = io_pool.tile([128, W], in_dt, tag="rec")
                    tar_t = io_pool.tile([128, W], in_dt, tag="tar")
                    nc.sync.dma_start(out=rec_t[:, :cw], in_=rec[:, c0:c0 + cw])
                    q = nc.scalar if DMA_SPLIT == 2 else nc.sync
                    q.dma_start(out=tar_t[:, :cw], in_=tar[:, c0:c0 + cw])
                    if stage == "dma":
                        continue
                d_t = tmp_pool.tile([128, W], d_dt, tag="d")
                sub_eng = nc.gpsimd if (SUB_ENGINE == "gp1" and ci == 1) \
                    else nc.vector
                sub_eng.tensor_sub(d_t[:, :cw], rec_t[:, :cw], tar_t[:, :cw])
                if stage == "sub":
                    continue
                v_t = tmp_pool.tile([128, W], bf16, tag="v")
                sq_eng = SQ_ENGINE if SQ_ENGINE != "mix" else \
                    ("dve" if ci % 2 else "act")
                if sq_eng == "dve":
                    # bf16 tensor_tensor runs in the DVE 2x mode
                    nc.vector.tensor_mul(v_t[:, :cw], d_t[:, :cw], d_t[:, :cw])
                else:
                    nc.scalar.activation(v_t[:, :cw], d_t[:, :cw], AT.Square)
                if stage == "sq":
                    continue
                if OUTT:
                    # stationary = v 128-col block, moving = ones: out [128,1]
                    # column sums land across all 128 partitions, so the
                    # PSUM->SBUF copy is 128-way parallel (free size W/128)
                    psT = psum_pool.tile([128, W // 128], f32, tag="ps")
                    for j in range(cw // 128):
                        nc.tensor.matmul(psT[:, j:j + 1],
                                         v_t[:, j * 128:(j + 1) * 128],
                                         ones[:], start=True, stop=True)
                    if stage == "mm":
                        continue
                    copy_out(ob[:, c0 // 128:(c0 + cw) // 128],
                             psT[:, :cw // 128])
                else:
                    ps = psum_pool.tile([1, W], f32, tag="ps")
                    for j in range(cw // TW):
                        nc.tensor.matmul(
                            ps[:, j * TW:(j + 1) * TW], ones[:],
                            v_t[:, j * TW:(j + 1) * TW],
                            start=True, stop=True)
                    if stage == "mm":
                        continue
                    copy_out(ob[:, c0:c0 + cw], ps[:, :cw])
        else:
            # input DMAs spread over the two hardware DGE queues (SP + Act)
            for t in range(n_tiles):
                rec_t = io_pool.tile([128, TW], in_dt, tag="rec")
                tar_t = io_pool.tile([128, TW], in_dt, tag="tar")
                rb = rec[t * 128:(t + 1) * 128, :]
                tb = tar[t * 128:(t + 1) * 128, :]
                if DMA_SPLIT == 1:
                    nc.sync.dma_start(out=rec_t[:], in_=rb)
                    nc.sync.dma_start(out=tar_t[:], in_=tb)
                else:
                    nc.sync.dma_start(out=rec_t[:], in_=rb)
                    nc.scalar.dma_start(out=tar_t[:], in_=tb)
                if stage == "dma":
                    continue
                d_t = tmp_pool.tile([128, TW], d_dt, tag="d")
                nc.vector.tensor_sub(d_t[:], rec_t[:], tar_t[:])
                if stage == "sub":
                    continue
                v_t = tmp_pool.tile([128, TW], bf16, tag="v")
                if SQ_ENGINE == "dve":
                    nc.vector.tensor_mul(v_t[:], d_t[:], d_t[:])
                else:
                    nc.scalar.activation(v_t[:], d_t[:], AT.Square)
                if stage == "sq":
                    continue
                ps = psum_pool.tile([1, TW], f32, tag="ps")
                nc.tensor.matmul(ps[:], ones[:], v_t[:], start=True, stop=True)
                if stage == "mm":
                    continue
                copy_out(ob[:, t * TW:(t + 1) * TW], ps[:])

        if stage == "full":
            nc.sync.dma_start(out=out[:], in_=ob[:])

    nc.compile()
    return nc


def kernel(reco, target, clabel, batch_index, num_batches, num_clusters):
    from concourse.bass_utils import run_bass_kernel_spmd

    B = int(num_batches)
    C = int(num_clusters)
    assert B == B_HC and C == C_HC, f"kernel hardcoded for B=32,C=128, got {B},{C}"
    nseg = B * C

    rec = np.ascontiguousarray(np.asarray(reco, dtype=np.float32).reshape(-1))
    tar = np.ascontiguousarray(np.asarray(target, dtype=np.float32).reshape(-1))
    cl = np.asarray(clabel).astype(np.int32).reshape(-1)
    bi = np.asarray(batch_index).astype(np.int32).reshape(-1)
    N = rec.shape[0]

    # host layout: group points by fused segment id (stable counting order)
    key = bi * np.int32(C) + cl                      # [N] in [0, 4096)
    order = np.argsort(key, kind="stable")
    key_s = key[order]
    counts = np.bincount(key, minlength=nseg).astype(np.int64)  # [B*C]
    w = (counts + 127) >> 7                          # columns per segment
    w_pc = w.reshape(NCORES, SEG_PER_CORE)
    colbase_pc = np.zeros((NCORES, SEG_PER_CORE), dtype=np.int64)
    colbase_pc[:, 1:] = np.cumsum(w_pc[:, :-1], axis=1)
    T_core = w_pc.sum(axis=1)                        # used cols per core
    n_tiles = max(1, int(-(-int(T_core.max()) // TW)))
    T = n_tiles * TW

    # destination slot of each (sorted) point: core, partition, column
    seg_start = np.zeros(nseg + 1, dtype=np.int64)
    seg_start[1:] = np.cumsum(counts)
    i_loc = np.arange(N, dtype=np.int64) - seg_start[key_s]
    p = i_loc & 127
    col = colbase_pc.reshape(-1)[key_s] + (i_loc >> 7)
    core = key_s >> np.int64(SEG_PER_CORE.bit_length() - 1)  # key_s // 512
    if LAYOUT == "big":
        # plain [128, T] per core
        dest = (core * 128 + p) * T + col
    else:
        # tile-major per-core layout: [n_tiles, 128, TW] flattened
        dest = (core * (n_tiles * 128) + (col // TW) * 128 + p) * TW + (col % TW)

    if IN_DTYPE == "f32":
        rec_buf = np.zeros(NCORES * n_tiles * 128 * TW, dtype=np.float32)
        tar_buf = np.zeros_like(rec_buf)
        rec_buf[dest] = rec[order]
        tar_buf[dest] = tar[order]
    else:
        import ml_dtypes
        cdt = ml_dtypes.bfloat16 if IN_DTYPE == "bf16" else ml_dtypes.float8_e4m3
        udt = np.uint16 if IN_DTYPE == "bf16" else np.uint8
        rec_buf = np.zeros(NCORES * n_tiles * 128 * TW, dtype=udt)
        tar_buf = np.zeros_like(rec_buf)
        rec_buf[dest] = rec[order].astype(cdt).view(udt)
        tar_buf[dest] = tar[order].astype(cdt).view(udt)
        rec_buf = rec_buf.view(cdt)
        tar_buf = tar_buf.view(cdt)
    in_shape = (128, T) if LAYOUT == "big" else (n_tiles * 128, TW)
    rec_buf = rec_buf.reshape(NCORES, *in_shape)
    tar_buf = tar_buf.reshape(NCORES, *in_shape)

    key_cache = (n_tiles, IN_DTYPE, DMA_SPLIT, SQ_ENGINE, COPY_ENGINE, LAYOUT, PSGROUP, OUTT, PREFETCH, SUB_ENGINE)
    if key_cache not in _prog_cache:
        _prog_cache[key_cache] = _build_program(n_tiles)
    nc = _prog_cache[key_cache]

    in_maps = [{"rec": rec_buf[m], "tar": tar_buf[m]} for m in range(NCORES)]
    _last_run["nc"] = nc
    _last_run["in_maps"] = in_maps
    _last_run["key"] = key_cache
    _last_run["n_tiles"] = n_tiles

    res = None
    last_err = None
    for _attempt in range(3):  # the device occasionally faults transiently
        try:
            res = run_bass_kernel_spmd(nc, in_maps, list(range(NCORES)))
            break
        except Exception as e:  # noqa: BLE001
            last_err = e
            import time as _time
            _time.sleep(2.0)
    if res is None:
        raise last_err

    # host: fold each segment's column sums, then the O(B*C) final reduction
    sums = np.zeros(nseg, dtype=np.float64)
    for m in range(NCORES):
        o = res.results[m]["out"].astype(np.float64)
        colsums = o.T.reshape(-1) if OUTT else o.reshape(-1)  # [T]
        cs = np.zeros(T + 1, dtype=np.float64)
        cs[1:] = np.cumsum(colsums)
        s0, s1 = colbase_pc[m], colbase_pc[m] + w_pc[m]
        sums[m * SEG_PER_CORE:(m + 1) * SEG_PER_CORE] = cs[s1] - cs[s0]

    counts_f = counts.astype(np.float64).reshape(B, C)
    sums2 = sums.reshape(B, C)
    present = counts_f > 0
    means = np.where(present, sums2 / np.where(present, counts_f, 1.0), 0.0)
    pmask = present.astype(np.float64)
    n_clusters_b = pmask.sum(axis=1)
    b_present = n_clusters_b > 0
    batch_loss = (means * pmask).sum(axis=1) / np.where(b_present, n_clusters_b, 1.0)
    n_b = b_present.sum()
    loss = np.where(b_present, batch_loss, 0.0).sum() / max(n_b, 1)
    return np.float32(loss)


def profile_hw(np_inputs=None, k1=4, k2=4004, pairs=12, verbose=False):
    """Measure steady-state HW ns per kernel iteration.

    Runs two hardware-loop variants (k1/k2 repeats of the full compute,
    Internal-DRAM inputs so no transfers) in interleaved pairs; the median
    of per-pair wall-clock differences divided by (k2-k1) cancels dispatch
    overhead and is robust to the time-shared device's slow patches.
    """
    import time
    from concourse.bass_utils import run_bass_kernel_spmd
    if not _last_run and np_inputs is not None:
        kernel(**np_inputs)
    n_tiles = _last_run["n_tiles"]

    ncs = {}
    for k in (k1, k2):
        ck = ("prof", n_tiles, k, "full", IN_DTYPE, DMA_SPLIT, SQ_ENGINE, COPY_ENGINE, LAYOUT, PSGROUP, OUTT, PREFETCH, SUB_ENGINE)
        if ck not in _prog_cache:
            _prog_cache[ck] = _build_program(n_tiles, repeat=k,
                                             internal_inputs=True)
        ncs[k] = _prog_cache[ck]

    def one(k):
        t0 = time.time()
        run_bass_kernel_spmd(ncs[k], [{} for _ in range(NCORES)],
                             list(range(NCORES)))
        return time.time() - t0

    one(k1)  # warm both NEFFs
    one(k2)
    t1s, t2s = [], []
    for _ in range(pairs):
        try:
            t1s.append(one(k1))
            t2s.append(one(k2))
        except Exception:  # transient device flake: skip pair
            time.sleep(2)
            continue
    if not t1s or not t2s:
        return float("nan")
    # min-min: the device is time-shared, so contention only ever adds
    # time; the minimum over repeats of each loop length estimates the
    # uncontended steady state.
    minmin = (min(t2s) - min(t1s)) / (k2 - k1) * 1e9
    if verbose:
        diffs = sorted((b - a) / (k2 - k1) * 1e9 for a, b in zip(t1s, t2s))
        print("pair diffs (ns/iter):", [f"{d:.0f}" for d in diffs])
        print(f"median-of-pairs: {diffs[len(diffs) // 2]:.0f} ns")
    return minmin


def profile_stages(np_inputs=None, k1=4, k2=1004, pairs=6):
    """Per-stage steady-state times (us): dma, +sub, +sq, +mm, full.

    Same interleaved-pair median methodology as profile_hw, per stage.
    """
    import time
    from concourse.bass_utils import run_bass_kernel_spmd
    if not _last_run and np_inputs is not None:
        kernel(**np_inputs)
    n_tiles = _last_run["n_tiles"]
    out = {}
    for stage in ("dma", "sub", "sq", "mm", "full"):
        ncs = {}
        for k in (k1, k2):
            ck = ("prof", n_tiles, k, stage, IN_DTYPE, DMA_SPLIT, SQ_ENGINE, COPY_ENGINE, LAYOUT, PSGROUP, OUTT, PREFETCH, SUB_ENGINE)
            if ck not in _prog_cache:
                _prog_cache[ck] = _build_program(n_tiles, repeat=k,
                                                 internal_inputs=True,
                                                 stage=stage)
            ncs[k] = _prog_cache[ck]

        def one(k):
            t0 = time.time()
            run_bass_kernel_spmd(ncs[k], [{} for _ in range(NCORES)],
                                 list(range(NCORES)))
            return time.time() - t0

        one(k1)
        one(k2)
        diffs = []
        for _ in range(pairs):
            try:
                ta = one(k1)
                tb = one(k2)
            except Exception:
                time.sleep(2)
                continue
            diffs.append((tb - ta) / (k2 - k1) * 1e6)
        diffs.sort()
        out[stage] = diffs[len(diffs) // 2] if diffs else float("nan")
    return out
